# revision 57
# baseline (speedup 1.0000x reference)
"""Trainium2 Bass kernel for the wf-psf TF_physical_poly_field forward model.

8 NeuronCores, data-parallel over the 32-star batch (4 stars/core).

Host prep (tiny, O(B*K) math):
  - exact-position match + polynomial features -> per-star coefficient row
    C[s, 0:87] over 87 basis maps (66 zernikes + 21 alpha-folded S rows).
  - basis maps pre-masked by the pupil obscuration, flattened C-order
    (f = i*256 + j) in fp16.
  - per-bin DFT tables: the reference zero-pads the 256x256 pupil to pN_j,
    FFTs, fftshifts and keeps the centred 96x96 crop.  |FFT|^2 is invariant
    to the zero-pad shift, so the crop equals A = E_j P E_j^T with
    E_j[f, y] = exp(-2pi i f y / pN_j), f in [-48, 48).  Stage 2 reuses
    column slices of the stage-1 tables (C, S, -S) as lhsT.
  - obscuration correction D_j = -E_j (1-obsc) E_j^T (from the fp16-rounded
    tables) as a single fp16 lhsT table, injected through identity-rhs
    matmuls; the device never multiplies by the mask: it computes
    P' = exp(i k opd_masked) (= P inside pupil, = 1 outside) and D cancels
    the outside region.

Device per core:
  1. opd: stream W in chunks over all three DMA queues (SP/Act hardware +
     Pool software DGE), 1024 transposed matmuls (lhsT = W[:, 128-block],
     rhs = C) accumulate opd directly as [128 pixel-partitions, (block, s)]
     in PSUM; four AP-permuted copies write the fp16 opd in the
     [j mod 128, (s, j-half, i)] layout the DFT stages consume.  The DFT
     tables ride the Pool queue behind the W chunks.
  2. software-pipelined bin loop (batched over the 4 stars, iteration jj
     emits stage1(jj-1), stage2(jj-2), elementwise(jj), tail(jj-2) so no
     in-order engine queue head-of-line blocks):
     - fp16-magic range reduction using only tensor_scalar/tensor_tensor
       (DVE 4x/2x fast modes; scalar_tensor_tensor has none), spread over
       Pool (r16) and DVE (nlr/th/nth/av);
     - two Sin activations (Act) give the cos/sin pupil planes;
     - stage-1/stage-2 DFT matmuls with D injection, PSUM->SBUF copies of
       U split Act/DVE;
     - Square (Act) + 3x3 sum-pool (Pool adds + pooling matmuls into spare
       PSUM columns of the A tile, flux totals via all-ones matmul),
       broadcast-scaled PSF accumulation on Pool.
"""

import numpy as np

import concourse.bacc as bacc
import concourse.tile as tile
from concourse import mybir
from concourse.bass_utils import run_bass_kernel_spmd

F32 = mybir.dt.float32
F16 = mybir.dt.float16
AF = mybir.ActivationFunctionType
ALU = mybir.AluOpType

# ---- static model configuration (mirrors the reference driver args) ----
BATCH = 32
N_ZKS_TOTAL = 66
N_ZKS_PARAM = 45
D_MAX = 2
D_MAX_NP = 5
OPD_DIM = 256
N_BINS = 20
OUTPUT_DIM = 32
OVERSAMPLING = 3.0
LAMBDAS = np.linspace(0.55, 0.9, N_BINS)
PHASE_NS = [int(2 * round(OPD_DIM * OVERSAMPLING * l / (2.0 * LAMBDAS[0])))
            for l in LAMBDAS]
N_CORES = 8
SPC = BATCH // N_CORES          # stars per core
KMAT = N_ZKS_TOTAL + 21         # 87 basis maps
CROP = 96                       # 96x96 centre crop of the FFT
NPIX = OPD_DIM * OPD_DIM

LAM32 = [float(np.float32(l)) for l in LAMBDAS]
KVAL = [float(np.float32(2.0 * np.pi) / np.float32(l)) for l in LAMBDAS]
MAGIC = 1536.0                  # fp16 round-to-int magic (quantum 1.0 there)
HALF_PI = float(np.pi / 2)

W_CHUNK = 2048                  # pixels per W streaming chunk
N_WCHUNK = NPIX // W_CHUNK      # 32


def _poly_pos_mat(positions, d_max):
    """fp32 Mendel-ordered polynomial position matrix, shape (n_poly, B)."""
    x = positions[:, 0] / np.float32(1000.0) * np.float32(2.0) - np.float32(1.0)
    y = positions[:, 1] / np.float32(1000.0) * np.float32(2.0) - np.float32(1.0)
    cols = []
    for d in range(d_max + 1):
        for p in range(d + 1):
            cols.append((x ** (d - p)) * (y ** p))
    return np.stack(cols, axis=0).astype(np.float32)


def _host_prep(positions, packed_SED_data, coeff_mat, alpha_mat, S_mat,
               zernike_maps, obscurations, obs_pos, zks_prior):
    pos = np.asarray(positions, np.float32)

    pm = _poly_pos_mat(pos, D_MAX)                          # (6, B)
    zk_param = (np.asarray(coeff_mat, np.float32) @ pm).T   # (B, 45)
    eq = (pos[:, None, :] == np.asarray(obs_pos, np.float32)[None, :, :]).all(-1)
    idx = eq.argmax(1)
    zks = np.asarray(zks_prior, np.float32)[idx].copy()     # (B, 66)
    zks[:, :N_ZKS_PARAM] += zk_param

    pm_np = _poly_pos_mat(pos, D_MAX_NP)                    # (21, B)
    beta = pm_np.T @ np.asarray(alpha_mat, np.float32)      # (B, 21)
    C = np.concatenate([zks, beta], axis=1)                 # (B, 87)

    obsc = np.asarray(obscurations, np.float32)
    W = np.concatenate([np.asarray(zernike_maps, np.float32),
                        np.asarray(S_mat, np.float32)], axis=0)
    Wm = W * obsc[None, :, :]
    Wt = np.ascontiguousarray(Wm.reshape(KMAT, NPIX)).astype(np.float16)

    f = np.arange(CROP, dtype=np.float64) - CROP // 2
    y = np.arange(OPD_DIM, dtype=np.float64)
    # stage-1 rhs tables: per coord-half, per bin 192 cols: taba = [C | -S]
    # (for Pr), tabb = [S | C] (for Pi) -> one N=192 matmul accumulates
    # [Ur | Ui].  Stage 2 reuses column slices of the same tables as lhsT:
    # C = taba[:, :96], S = tabb[:, :96], -S = taba[:, 96:192].
    taba = np.empty((2, 128, N_BINS * 192), np.float16)
    tabb = np.empty_like(taba)
    # per-bin obscuration correction D as hi+lo fp16 lhsT tables, injected
    # through identity-rhs matmuls
    dthi = np.zeros((CROP, N_BINS * 2 * CROP), np.float16)
    m1 = (1.0 - obsc).astype(np.float64)
    for j in range(N_BINS):
        ang = 2.0 * np.pi * np.outer(y, f) / PHASE_NS[j]    # (256, 96)
        c16 = np.cos(ang).astype(np.float16)
        s16 = np.sin(ang).astype(np.float16)
        for t in range(2):
            rows = slice(t * 128, (t + 1) * 128)
            taba[t, :, j * 192:j * 192 + 96] = c16[rows]
            taba[t, :, j * 192 + 96:(j + 1) * 192] = -s16[rows]
            tabb[t, :, j * 192:j * 192 + 96] = s16[rows]
            tabb[t, :, j * 192 + 96:(j + 1) * 192] = c16[rows]
        Eh = (c16.astype(np.float64) - 1j * s16.astype(np.float64)).T  # (96,256)
        D = -(Eh @ m1 @ Eh.T)                               # (96, 96) complex
        for part, Dp in ((0, D.real), (1, D.imag)):
            col = (2 * j + part) * CROP
            dthi[:, col:col + CROP] = Dp.T.astype(np.float16)

    i4 = np.zeros((CROP, SPC * CROP), np.float16)
    for s in range(SPC):
        i4[:, s * CROP:(s + 1) * CROP] = np.eye(CROP, dtype=np.float16)

    # partition-pooling matmul (3->1) and all-ones totals matmul
    qt32 = np.zeros((CROP, 32), np.float32)
    for k in range(CROP):
        qt32[k, k // 3] = 1.0
    ones96 = np.ones((CROP, 32), np.float32)

    sed = np.asarray(packed_SED_data, np.float32)[:, :, 2]  # (B, 20)
    return (C, Wt, taba, tabb, dthi, i4, qt32, ones96, sed)


def _build_nc(repeat=1):
    nc = bacc.Bacc("TRN2", target_bir_lowering=False)

    cmat = nc.dram_tensor("cmat", [KMAT, SPC], F16, kind="ExternalInput")
    wmat = nc.dram_tensor("wmat", [KMAT, NPIX], F16, kind="ExternalInput")
    taba_d = nc.dram_tensor("taba", [2, 128, N_BINS * 192], F16,
                            kind="ExternalInput")
    tabb_d = nc.dram_tensor("tabb", [2, 128, N_BINS * 192], F16,
                            kind="ExternalInput")
    dthi_d = nc.dram_tensor("dthi", [CROP, N_BINS * 2 * CROP], F16,
                            kind="ExternalInput")
    i4_d = nc.dram_tensor("i4", [CROP, SPC * CROP], F16, kind="ExternalInput")
    qt32_d = nc.dram_tensor("qt32", [CROP, 32], F32, kind="ExternalInput")
    ones_d = nc.dram_tensor("ones96", [CROP, 32], F32, kind="ExternalInput")
    sed_d = nc.dram_tensor("sed", [32, SPC * N_BINS], F32, kind="ExternalInput")
    psf_out = nc.dram_tensor("psf_out", [SPC, OUTPUT_DIM, OUTPUT_DIM], F32,
                             kind="ExternalOutput")

    with tile.TileContext(nc) as tc:
        with tc.tile_pool(name="const", bufs=1) as cpool:
            halfpi = cpool.tile([128, 1], F32)
            nc.gpsimd.memset(halfpi[:], HALF_PI)
            c_sb = cpool.tile([KMAT, SPC], F16)
            nc.sync.dma_start(c_sb[:], cmat[:])
            taba_sb = [cpool.tile([128, N_BINS * 192], F16, name=f"taba{t}",
                                  tag=f"ta{t}") for t in range(2)]
            tabb_sb = [cpool.tile([128, N_BINS * 192], F16, name=f"tabb{t}",
                                  tag=f"tb{t}") for t in range(2)]
            dthi_sb = cpool.tile([CROP, N_BINS * 2 * CROP], F16)
            i4_sb = cpool.tile([CROP, SPC * CROP], F16)
            nc.gpsimd.dma_start(i4_sb[:], i4_d[:])
            qt32_sb = cpool.tile([CROP, 32], F32)
            ones_sb = cpool.tile([CROP, 32], F32)
            sed_sb = cpool.tile([32, SPC * N_BINS], F32)
            opd16 = cpool.tile([128, SPC * 512], F16)   # [j%128, (s, t, i)]
            psf_all = cpool.tile([32, SPC * 32], F32)
            nc.gpsimd.memset(psf_all[:], 0.0)

            import contextlib
            rep_ctx = (tc.For_i(0, repeat, 1, hint_engines=tuple(nc.engines))
                       if repeat > 1 else contextlib.nullcontext())
            with rep_ctx:
                # ---- opd phase: 512 transposed matmuls into 4 PSUM banks ----
                with tc.tile_pool(name="wpool", bufs=16) as wpool, \
                     tc.tile_pool(name="opd_ps", bufs=1, space="PSUM") as opd_ps:
                    ops = opd_ps.tile([128, 2048], F32, tag="ops")
                    # W chunks split 6/6/4 over the SP/Act/Pool DMA
                    # queues, interleaved so ring-slot reuse never stalls
                    # behind a slower queue
                    wc_eng = tuple((nc.sync, nc.scalar, nc.gpsimd, nc.sync,
                                    nc.scalar, nc.sync, nc.gpsimd, nc.scalar)[i % 8]
                                   for i in range(N_WCHUNK))
                    for ci in range(N_WCHUNK):
                        wc = wpool.tile([KMAT, W_CHUNK], F16, tag="wc")
                        wc_eng[ci].dma_start(
                            wc[:], wmat[:, ci * W_CHUNK:(ci + 1) * W_CHUNK])
                        for b in range(W_CHUNK // 128):
                            m = ci * (W_CHUNK // 128) + b
                            nc.tensor.matmul(ops[:, 4 * m:4 * m + 4],
                                             wc[:, 128 * b:128 * (b + 1)],
                                             c_sb[:], start=True, stop=True)
                    # tables ride the Pool queue behind the W chunks (stage
                    # 1/2 need them only once the pipeline fills); keeping
                    # them off SP/Act stops the scheduler from reordering
                    # them ahead of the W stream
                    for t in range(2):
                        nc.gpsimd.dma_start(taba_sb[t][:], taba_d[t])
                        nc.gpsimd.dma_start(tabb_sb[t][:], tabb_d[t])
                    nc.sync.dma_start(dthi_sb[:], dthi_d[:])
                    nc.scalar.dma_start(qt32_sb[:], qt32_d[:])
                    nc.scalar.dma_start(ones_sb[:], ones_d[:])
                    nc.scalar.dma_start(sed_sb[:], sed_d[:])
                    # PSUM [p, (i, t, s)] -> SBUF [p, (s, t, i)] fp16, one
                    # AP-permuted copy per bank spread across engines
                    o16v = opd16[:].rearrange("p (s t i) -> p s t i", s=SPC, t=2)
                    for q in range(4):
                        src = ops[:, 512 * q:512 * (q + 1)].rearrange(
                            "p (i t s) -> p s t i", t=2, s=SPC)
                        dst = o16v[:, :, :, 64 * q:64 * (q + 1)]
                        if q % 2 == 0:
                            nc.vector.tensor_copy(dst, src)
                        else:
                            nc.scalar.copy(dst, src)

                # ---- main loop, software-pipelined over bins:
                # iteration jj emits elementwise(jj), stage1(jj-1),
                # stage2+tail(jj-2) so no engine queue head-of-line blocks
                with tc.tile_pool(name="elw", bufs=2) as elw, \
                     tc.tile_pool(name="usb", bufs=2) as usbp, \
                     tc.tile_pool(name="sqp", bufs=2) as sqp, \
                     tc.tile_pool(name="tailp", bufs=2) as tailp, \
                     tc.tile_pool(name="u_ps", bufs=1, space="PSUM") as u_ps, \
                     tc.tile_pool(name="a_ps", bufs=2, space="PSUM") as a_ps:
                    state = {}

                    def emit_elw(j):
                        lam = LAM32[j]
                        kj = KVAL[j]
                        # fp16-magic range reduction, batched over the 4
                        # stars: th = opd - lam*round(opd/lam) in
                        # [-lam/2, lam/2]; av = |th| lets the cos plane use
                        # the same [-pi, pi] Sin table
                        # ts/tt ops run DVE fast modes (4x/2x); stt has
                        # none.  nlr/nth share the r16/rr tag rings.
                        eng0 = nc.vector if j < 3 else nc.gpsimd
                        r16 = elw.tile([128, SPC * 512], F16, tag="r16")
                        eng0.tensor_scalar(r16[:], opd16[:], 1.0 / lam,
                                           MAGIC, op0=ALU.mult, op1=ALU.add)
                        pp = elw.tile([128, SPC * 1024], F16,
                                      name=f"pp_{j}", tag="pp")
                        nlr = elw.tile([128, SPC * 512], F16, tag="nlr")
                        nc.vector.tensor_scalar(nlr[:], r16[:], -lam,
                                                lam * MAGIC, op0=ALU.mult,
                                                op1=ALU.add)
                        th = elw.tile([128, SPC * 512], F16, tag="th")
                        nc.vector.tensor_tensor(th[:], opd16[:], nlr[:],
                                                op=ALU.add)
                        nth = elw.tile([128, SPC * 512], F16, tag="rr")
                        nc.vector.tensor_scalar(nth[:], th[:],
                                                -1.0, None, op0=ALU.mult)
                        av = elw.tile([128, SPC * 512], F16, tag="av")
                        nc.vector.tensor_tensor(av[:], th[:], nth[:],
                                                op=ALU.max)
                        nc.scalar.activation(pp[:, 0:2048], av[:], AF.Sin,
                                             bias=halfpi[:], scale=-kj)
                        nc.scalar.activation(pp[:, 2048:4096], th[:], AF.Sin,
                                             bias=0.0, scale=kj)
                        state[j] = {"pp": pp}

                    def emit_stage1(j):
                        # stage 1: U = P E^T per star into u_ps; star s at
                        # column 512*(s//2) + 192*(s%2) (bank-aligned)
                        pp = state[j]["pp"]
                        ups = [u_ps.tile([128, 1024], F32,
                                         name=f"ups{_t}_{j}", tag=f"ups{_t}")
                               for _t in range(2)]
                        usb = [usbp.tile([128, SPC * 192], F16,
                                         name=f"usb{t}_{j}", tag=f"u{t}")
                               for t in range(2)]
                        s1 = slice(j * 192, (j + 1) * 192)
                        for s in range(SPC):
                            u0 = 512 * (s // 2) + 192 * (s % 2)
                            for xt in range(2):
                                out = ups[xt][:, u0:u0 + 192]
                                for yi, yt in enumerate((0, 1)):
                                    prs = pp[:, 512 * s + 256 * yt + 128 * xt:
                                             512 * s + 256 * yt + 128 * (xt + 1)]
                                    pis = pp[:, 2048 + 512 * s + 256 * yt + 128 * xt:
                                             2048 + 512 * s + 256 * yt + 128 * (xt + 1)]
                                    nc.tensor.matmul(out, prs,
                                                     taba_sb[yt][:, s1],
                                                     start=(yi == 0),
                                                     stop=False)
                                    nc.tensor.matmul(out, pis,
                                                     tabb_sb[yt][:, s1],
                                                     start=False,
                                                     stop=(yi == 1))
                        # PSUM->SBUF fp16 copies: usb1 on DVE, usb0 split
                        # 2/3 Act + 1/3 DVE to balance engine load
                        src0 = ups[0][:].rearrange(
                            "p (s2 q) -> p s2 q", s2=2)
                        dst0 = usb[0][:].rearrange(
                            "p (s2 r) -> p s2 r", s2=2)
                        nc.scalar.copy(dst0[:, :, 0:288], src0[:, :, 0:288])
                        nc.vector.tensor_copy(dst0[:, :, 288:384],
                                              src0[:, :, 288:384])
                        src1 = ups[1][:].rearrange(
                            "p (s2 q) -> p s2 q", s2=2)[:, :, 0:384]
                        dst1 = usb[1][:].rearrange(
                            "p (s2 r) -> p s2 r", s2=2)
                        nc.vector.tensor_copy(dst1, src1)
                        state[j]["usb"] = usb

                    def emit_stage2(j):
                        # stage 2: A = E U + D; lhsT tables are column
                        # slices of taba/tabb (C, S, -S); D injected via
                        # identity-rhs matmuls from the hi/lo fp16 tables
                        usb = state[j]["usb"]
                        a_all = a_ps.tile([128, SPC * 256], F32,
                                          name=f"a_{j}", tag="a")
                        ctab = [taba_sb[t][:, j * 192:j * 192 + 96]
                                for t in range(2)]
                        stab = [tabb_sb[t][:, j * 192:j * 192 + 96]
                                for t in range(2)]
                        nstab = [taba_sb[t][:, j * 192 + 96:(j + 1) * 192]
                                 for t in range(2)]
                        dre = slice((2 * j) * CROP, (2 * j + 1) * CROP)
                        dim = slice((2 * j + 1) * CROP, (2 * j + 2) * CROP)
                        for s in range(SPC):
                            a_s = a_all[0:CROP, 256 * s:256 * s + 192]
                            are = a_all[0:CROP, 256 * s:256 * s + 96]
                            aim = a_all[0:CROP, 256 * s + 96:256 * s + 192]
                            uboth = [usb[xt][:, 192 * s:192 * (s + 1)]
                                     for xt in range(2)]
                            ur = [usb[xt][:, 192 * s:192 * s + 96]
                                  for xt in range(2)]
                            ui = [usb[xt][:, 192 * s + 96:192 * (s + 1)]
                                  for xt in range(2)]
                            islc = i4_sb[:, s * CROP:(s + 1) * CROP]
                            nc.tensor.matmul(a_s, ctab[0], uboth[0],
                                             start=True, stop=False)
                            nc.tensor.matmul(a_s, ctab[1], uboth[1],
                                             start=False, stop=False)
                            nc.tensor.matmul(are, stab[0], ui[0],
                                             start=False, stop=False)
                            nc.tensor.matmul(are, stab[1], ui[1],
                                             start=False, stop=False)
                            nc.tensor.matmul(aim, nstab[0], ur[0],
                                             start=False, stop=False)
                            nc.tensor.matmul(aim, nstab[1], ur[1],
                                             start=False, stop=False)
                            nc.tensor.matmul(are, dthi_sb[:, dre], islc,
                                             start=False, stop=False)
                            nc.tensor.matmul(aim, dthi_sb[:, dim], islc,
                                             start=False, stop=True)
                        state[j]["a"] = a_all

                    def emit_tail(j):
                        # ---- bin tail (batched over the 4 stars) ----
                        a_all = state[j]["a"]
                        sq = sqp.tile([CROP, SPC * 192], F32, tag="sq")
                        av4 = a_all[0:CROP, :].rearrange("p (s g) -> p s g",
                                                         g=256)
                        nc.scalar.activation(
                            sq[:].rearrange("p (s g) -> p s g", g=192),
                            av4[:, :, 0:192], AF.Square)
                        ps_all = sqp.tile([CROP, SPC * 96], F32, tag="ps")
                        sq4 = sq[:].rearrange("p (s h g) -> p s h g", h=2, g=96)
                        nc.gpsimd.tensor_tensor(
                            ps_all[:].rearrange("p (s g) -> p s g", g=96),
                            sq4[:, :, 0, :], sq4[:, :, 1, :], op=ALU.add)
                        ps1 = tailp.tile([CROP, 132], F32, tag="ps1")
                        pv = ps_all[:].rearrange("p (s q c) -> p s q c",
                                                 q=32, c=3)
                        t1 = tailp.tile([CROP, 128], F32, tag="t1")
                        nc.gpsimd.tensor_tensor(
                            t1[:].rearrange("p (s q) -> p s q", q=32),
                            pv[:, :, :, 0], pv[:, :, :, 1], op=ALU.add)
                        nc.gpsimd.tensor_tensor(
                            ps1[:, 0:128].rearrange("p (s q) -> p s q", q=32),
                            t1[:].rearrange("p (s q) -> p s q", q=32),
                            pv[:, :, :, 2], op=ALU.add)
                        nc.vector.tensor_reduce(
                            ps1[:, 128:132],
                            ps1[:, 0:128].rearrange("p (s q) -> p s q", s=SPC),
                            axis=mybir.AxisListType.X, op=ALU.add)
                        # pooling + totals matmuls land in the spare PSUM
                        # columns of this bin's a tile (cols 192.. of the
                        # star blocks)
                        plp = [a_all[0:32, 192:256], a_all[0:32, 448:512]]
                        totp = a_all[0:32, 704:704 + SPC]
                        nc.tensor.matmul(plp[0], qt32_sb[:], ps1[:, 0:64],
                                         start=True, stop=True)
                        nc.tensor.matmul(plp[1], qt32_sb[:], ps1[:, 64:128],
                                         start=True, stop=True)
                        nc.tensor.matmul(totp, ones_sb[:], ps1[:, 128:132],
                                         start=True, stop=True)
                        plsb = tailp.tile([32, 128], F32, tag="plsb")
                        psrc = a_all[0:32, 192:704].rearrange(
                            "p (h c) -> p h c", c=256)[:, :, 0:64]
                        nc.vector.tensor_copy(
                            plsb[:].rearrange("p (h c) -> p h c", c=64), psrc)
                        rcp = tailp.tile([32, SPC], F32, tag="rcp")
                        nc.vector.reciprocal(rcp[:], totp)
                        scl = tailp.tile([32, SPC], F32, tag="scl")
                        nc.gpsimd.tensor_tensor(
                            scl[:], rcp[:], sed_sb[:, j * SPC:(j + 1) * SPC],
                            op=ALU.mult)
                        sclb = scl[:].rearrange(
                            "p (s o) -> p s o", o=1).broadcast_to((32, SPC, 32))
                        tmp = tailp.tile([32, SPC * 32], F32, tag="tmp")
                        nc.gpsimd.tensor_tensor(
                            tmp[:].rearrange("p (s q) -> p s q", q=32),
                            plsb[:].rearrange("p (s q) -> p s q", q=32),
                            sclb, op=ALU.mult)
                        nc.gpsimd.tensor_tensor(psf_all[:], psf_all[:],
                                                tmp[:], op=ALU.add)
                        del state[j]

                    for jj in range(N_BINS + 2):
                        if 1 <= jj < N_BINS + 1:
                            emit_stage1(jj - 1)
                        if 2 <= jj:
                            emit_stage2(jj - 2)
                        if jj < N_BINS:
                            emit_elw(jj)
                        if 2 <= jj:
                            emit_tail(jj - 2)

                    for s in range(SPC):
                        nc.gpsimd.dma_start(psf_out[s],
                                            psf_all[:, 32 * s:32 * (s + 1)])

    nc.compile()
    return nc


_NC_CACHE = []


def _make_in_maps(prep):
    (C, Wt, taba, tabb, dthi, i4, qt32, ones96, sed) = prep
    shared = {
        "wmat": Wt, "taba": taba, "tabb": tabb, "dthi": dthi,
        "i4": i4, "qt32": qt32, "ones96": ones96,
    }
    in_maps = []
    for c in range(N_CORES):
        sl = slice(c * SPC, (c + 1) * SPC)
        sed_row = np.broadcast_to(
            sed[sl].T.reshape(1, N_BINS * SPC), (32, N_BINS * SPC))
        sed_row = np.ascontiguousarray(sed_row).astype(np.float32)
        in_maps.append(dict(
            shared,
            cmat=np.ascontiguousarray(C[sl].T).astype(np.float16),
            sed=sed_row,
        ))
    return in_maps


def kernel(**inputs):
    in_maps = _make_in_maps(_host_prep(**inputs))

    if not _NC_CACHE:
        _NC_CACHE.append(_build_nc())
    nc = _NC_CACHE[0]

    res = run_bass_kernel_spmd(nc, in_maps, core_ids=list(range(N_CORES)))
    out = np.concatenate([r["psf_out"] for r in res.results], axis=0)
    return out.astype(np.float32)


# revision 59
# speedup vs baseline: 2.3631x; 2.3631x over previous
"""Trainium2 Bass kernel for the wf-psf TF_physical_poly_field forward model.

8 NeuronCores, data-parallel over the 32-star batch (4 stars/core).

Host prep (tiny, O(B*K) math):
  - exact-position match + polynomial features -> per-star coefficient row
    C[s, 0:87] over 87 basis maps (66 zernikes + 21 alpha-folded S rows).
  - basis maps pre-masked by the pupil obscuration, flattened C-order
    (f = i*256 + j) in fp16.
  - per-bin DFT tables: the reference zero-pads the 256x256 pupil to pN_j,
    FFTs, fftshifts and keeps the centred 96x96 crop.  |FFT|^2 is invariant
    to the zero-pad shift, so the crop equals A = E_j P E_j^T with
    E_j[f, y] = exp(-2pi i f y / pN_j), f in [-48, 48).  Stage 2 reuses
    column slices of the stage-1 tables (C, S, -S) as lhsT.
  - obscuration correction D_j = -E_j (1-obsc) E_j^T (from the fp16-rounded
    tables) as a single fp16 lhsT table, injected through identity-rhs
    matmuls; the device never multiplies by the mask: it computes
    P' = exp(i k opd_masked) (= P inside pupil, = 1 outside) and D cancels
    the outside region.

Device per core:
  1. opd: stream W in chunks over all three DMA queues (SP/Act hardware +
     Pool software DGE), 1024 transposed matmuls (lhsT = W[:, 128-block],
     rhs = C) accumulate opd directly as [128 pixel-partitions, (block, s)]
     in PSUM; four AP-permuted copies write the fp16 opd in the
     [j mod 128, (s, j-half, i)] layout the DFT stages consume.  The DFT
     tables ride the Pool queue behind the W chunks.
  2. software-pipelined bin loop (batched over the 4 stars, iteration jj
     emits stage1(jj-1), stage2(jj-2), elementwise(jj), tail(jj-2) so no
     in-order engine queue head-of-line blocks):
     - fp16-magic range reduction using only tensor_scalar/tensor_tensor
       (DVE 4x/2x fast modes; scalar_tensor_tensor has none), spread over
       Pool (r16) and DVE (nlr/th/nth/av);
     - two Sin activations (Act) give the cos/sin pupil planes;
     - stage-1/stage-2 DFT matmuls with D injection, PSUM->SBUF copies of
       U split Act/DVE;
     - Square (Act) + 3x3 sum-pool (Pool adds + pooling matmuls into spare
       PSUM columns of the A tile, flux totals via all-ones matmul),
       broadcast-scaled PSF accumulation on Pool.
"""

import numpy as np

import concourse.bacc as bacc
import concourse.tile as tile
from concourse import mybir
from concourse.bass_utils import run_bass_kernel_spmd

F32 = mybir.dt.float32
F16 = mybir.dt.float16
AF = mybir.ActivationFunctionType
ALU = mybir.AluOpType

# ---- static model configuration (mirrors the reference driver args) ----
BATCH = 32
N_ZKS_TOTAL = 66
N_ZKS_PARAM = 45
D_MAX = 2
D_MAX_NP = 5
OPD_DIM = 256
N_BINS = 20
OUTPUT_DIM = 32
OVERSAMPLING = 3.0
LAMBDAS = np.linspace(0.55, 0.9, N_BINS)
PHASE_NS = [int(2 * round(OPD_DIM * OVERSAMPLING * l / (2.0 * LAMBDAS[0])))
            for l in LAMBDAS]
N_CORES = 8
SPC = BATCH // N_CORES          # stars per core
KMAT = N_ZKS_TOTAL + 21         # 87 basis maps
CROP = 96                       # 96x96 centre crop of the FFT
NPIX = OPD_DIM * OPD_DIM

LAM32 = [float(np.float32(l)) for l in LAMBDAS]
KVAL = [float(np.float32(2.0 * np.pi) / np.float32(l)) for l in LAMBDAS]
MAGIC = 1536.0                  # fp16 round-to-int magic (quantum 1.0 there)
HALF_PI = float(np.pi / 2)

W_CHUNK = 2048                  # pixels per W streaming chunk
N_WCHUNK = NPIX // W_CHUNK      # 32


def _poly_pos_mat(positions, d_max):
    """fp32 Mendel-ordered polynomial position matrix, shape (n_poly, B)."""
    x = positions[:, 0] / np.float32(1000.0) * np.float32(2.0) - np.float32(1.0)
    y = positions[:, 1] / np.float32(1000.0) * np.float32(2.0) - np.float32(1.0)
    cols = []
    for d in range(d_max + 1):
        for p in range(d + 1):
            cols.append((x ** (d - p)) * (y ** p))
    return np.stack(cols, axis=0).astype(np.float32)


def _host_prep(positions, packed_SED_data, coeff_mat, alpha_mat, S_mat,
               zernike_maps, obscurations, obs_pos, zks_prior):
    pos = np.asarray(positions, np.float32)

    pm = _poly_pos_mat(pos, D_MAX)                          # (6, B)
    zk_param = (np.asarray(coeff_mat, np.float32) @ pm).T   # (B, 45)
    eq = (pos[:, None, :] == np.asarray(obs_pos, np.float32)[None, :, :]).all(-1)
    idx = eq.argmax(1)
    zks = np.asarray(zks_prior, np.float32)[idx].copy()     # (B, 66)
    zks[:, :N_ZKS_PARAM] += zk_param

    pm_np = _poly_pos_mat(pos, D_MAX_NP)                    # (21, B)
    beta = pm_np.T @ np.asarray(alpha_mat, np.float32)      # (B, 21)
    C = np.concatenate([zks, beta], axis=1)                 # (B, 87)

    obsc = np.asarray(obscurations, np.float32)
    W = np.concatenate([np.asarray(zernike_maps, np.float32),
                        np.asarray(S_mat, np.float32)], axis=0)
    Wm = W * obsc[None, :, :]
    Wt = np.ascontiguousarray(Wm.reshape(KMAT, NPIX)).astype(np.float16)

    f = np.arange(CROP, dtype=np.float64) - CROP // 2
    y = np.arange(OPD_DIM, dtype=np.float64)
    # stage-1 rhs tables: per coord-half, per bin 192 cols: taba = [C | -S]
    # (for Pr), tabb = [S | C] (for Pi) -> one N=192 matmul accumulates
    # [Ur | Ui].  Stage 2 reuses column slices of the same tables as lhsT:
    # C = taba[:, :96], S = tabb[:, :96], -S = taba[:, 96:192].
    taba = np.empty((2, 128, N_BINS * 192), np.float16)
    tabb = np.empty_like(taba)
    cpad = np.zeros((2, 128, N_BINS * 128), np.float16)
    spad = np.zeros_like(cpad)
    nspad = np.zeros_like(cpad)
    # per-bin obscuration correction D as hi+lo fp16 lhsT tables, injected
    # through identity-rhs matmuls
    dthi = np.zeros((CROP, N_BINS * 2 * CROP), np.float16)
    m1 = (1.0 - obsc).astype(np.float64)
    for j in range(N_BINS):
        ang = 2.0 * np.pi * np.outer(y, f) / PHASE_NS[j]    # (256, 96)
        c16 = np.cos(ang).astype(np.float16)
        s16 = np.sin(ang).astype(np.float16)
        for t in range(2):
            rows = slice(t * 128, (t + 1) * 128)
            taba[t, :, j * 192:j * 192 + 96] = c16[rows]
            taba[t, :, j * 192 + 96:(j + 1) * 192] = -s16[rows]
            tabb[t, :, j * 192:j * 192 + 96] = s16[rows]
            tabb[t, :, j * 192 + 96:(j + 1) * 192] = c16[rows]
            cpad[t, :, j * 128:j * 128 + 96] = c16[rows]
            spad[t, :, j * 128:j * 128 + 96] = s16[rows]
            nspad[t, :, j * 128:j * 128 + 96] = -s16[rows]
        Eh = (c16.astype(np.float64) - 1j * s16.astype(np.float64)).T  # (96,256)
        D = -(Eh @ m1 @ Eh.T)                               # (96, 96) complex
        for part, Dp in ((0, D.real), (1, D.imag)):
            col = (2 * j + part) * CROP
            dthi[:, col:col + CROP] = Dp.T.astype(np.float16)

    i4 = np.zeros((CROP, SPC * CROP), np.float16)
    for s in range(SPC):
        i4[:, s * CROP:(s + 1) * CROP] = np.eye(CROP, dtype=np.float16)

    # partition-pooling matmul (3->1) and all-ones totals matmul
    qt32 = np.zeros((CROP, 32), np.float32)
    for k in range(CROP):
        qt32[k, k // 3] = 1.0
    ones96 = np.ones((CROP, 32), np.float32)

    sed = np.asarray(packed_SED_data, np.float32)[:, :, 2]  # (B, 20)
    return (C, Wt, taba, tabb, cpad, spad, nspad, dthi, i4, qt32, ones96, sed)


def _build_nc(repeat=1):
    nc = bacc.Bacc("TRN2", target_bir_lowering=False)

    cmat = nc.dram_tensor("cmat", [KMAT, SPC], F16, kind="ExternalInput")
    wmat = nc.dram_tensor("wmat", [KMAT, NPIX], F16, kind="ExternalInput")
    taba_d = nc.dram_tensor("taba", [2, 128, N_BINS * 192], F16,
                            kind="ExternalInput")
    tabb_d = nc.dram_tensor("tabb", [2, 128, N_BINS * 192], F16,
                            kind="ExternalInput")
    cpad_d = nc.dram_tensor("cpad", [2, 128, N_BINS * 128], F16,
                            kind="ExternalInput")
    spad_d = nc.dram_tensor("spad", [2, 128, N_BINS * 128], F16,
                            kind="ExternalInput")
    nspad_d = nc.dram_tensor("nspad", [2, 128, N_BINS * 128], F16,
                             kind="ExternalInput")
    dthi_d = nc.dram_tensor("dthi", [CROP, N_BINS * 2 * CROP], F16,
                            kind="ExternalInput")
    i4_d = nc.dram_tensor("i4", [CROP, SPC * CROP], F16, kind="ExternalInput")
    qt32_d = nc.dram_tensor("qt32", [CROP, 32], F32, kind="ExternalInput")
    ones_d = nc.dram_tensor("ones96", [CROP, 32], F32, kind="ExternalInput")
    sed_d = nc.dram_tensor("sed", [32, SPC * N_BINS], F32, kind="ExternalInput")
    psf_out = nc.dram_tensor("psf_out", [SPC, OUTPUT_DIM, OUTPUT_DIM], F32,
                             kind="ExternalOutput")

    with tile.TileContext(nc) as tc:
        with tc.tile_pool(name="const", bufs=1) as cpool:
            halfpi = cpool.tile([128, 1], F32)
            nc.gpsimd.memset(halfpi[:], HALF_PI)
            c_sb = cpool.tile([KMAT, SPC], F16)
            nc.sync.dma_start(c_sb[:], cmat[:])
            taba_sb = [cpool.tile([128, N_BINS * 192], F16, name=f"taba{t}",
                                  tag=f"ta{t}") for t in range(2)]
            tabb_sb = [cpool.tile([128, N_BINS * 192], F16, name=f"tabb{t}",
                                  tag=f"tb{t}") for t in range(2)]
            cpad_sb = [cpool.tile([128, N_BINS * 128], F16, name=f"cpad{t}",
                                  tag=f"cp{t}") for t in range(2)]
            spad_sb = [cpool.tile([128, N_BINS * 128], F16, name=f"spad{t}",
                                  tag=f"sp{t}") for t in range(2)]
            nspad_sb = [cpool.tile([128, N_BINS * 128], F16, name=f"nspad{t}",
                                   tag=f"nsp{t}") for t in range(2)]
            dthi_sb = cpool.tile([CROP, N_BINS * 2 * CROP], F16)
            i4_sb = cpool.tile([CROP, SPC * CROP], F16)
            nc.gpsimd.dma_start(i4_sb[:], i4_d[:])
            qt32_sb = cpool.tile([CROP, 32], F32)
            ones_sb = cpool.tile([CROP, 32], F32)
            sed_sb = cpool.tile([32, SPC * N_BINS], F32)
            opd16 = cpool.tile([128, SPC * 512], F16)   # [j%128, (s, t, i)]
            psf_all = cpool.tile([32, SPC * 32], F32)
            nc.gpsimd.memset(psf_all[:], 0.0)

            import contextlib
            rep_ctx = (tc.For_i(0, repeat, 1, hint_engines=tuple(nc.engines))
                       if repeat > 1 else contextlib.nullcontext())
            with rep_ctx:
                # ---- opd phase: 512 transposed matmuls into 4 PSUM banks ----
                with tc.tile_pool(name="wpool", bufs=16) as wpool, \
                     tc.tile_pool(name="opd_ps", bufs=1, space="PSUM") as opd_ps:
                    ops = opd_ps.tile([128, 2048], F32, tag="ops")
                    # W chunks split 6/6/4 over the SP/Act/Pool DMA
                    # queues, interleaved so ring-slot reuse never stalls
                    # behind a slower queue
                    wc_eng = tuple((nc.sync, nc.scalar, nc.gpsimd, nc.sync,
                                    nc.scalar, nc.sync, nc.gpsimd, nc.scalar)[i % 8]
                                   for i in range(N_WCHUNK))
                    for ci in range(N_WCHUNK):
                        wc = wpool.tile([KMAT, W_CHUNK], F16, tag="wc")
                        wc_eng[ci].dma_start(
                            wc[:], wmat[:, ci * W_CHUNK:(ci + 1) * W_CHUNK])
                        for b in range(W_CHUNK // 128):
                            m = ci * (W_CHUNK // 128) + b
                            nc.tensor.matmul(ops[:, 4 * m:4 * m + 4],
                                             wc[:, 128 * b:128 * (b + 1)],
                                             c_sb[:], start=True, stop=True)
                    # tables ride the Pool queue behind the W chunks (stage
                    # 1/2 need them only once the pipeline fills); keeping
                    # them off SP/Act stops the scheduler from reordering
                    # them ahead of the W stream
                    for t in range(2):
                        nc.gpsimd.dma_start(taba_sb[t][:], taba_d[t])
                        nc.gpsimd.dma_start(tabb_sb[t][:], tabb_d[t])
                    for t in range(2):
                        nc.gpsimd.dma_start(cpad_sb[t][:], cpad_d[t])
                        nc.gpsimd.dma_start(spad_sb[t][:], spad_d[t])
                        nc.gpsimd.dma_start(nspad_sb[t][:], nspad_d[t])
                    nc.sync.dma_start(dthi_sb[:], dthi_d[:])
                    nc.scalar.dma_start(qt32_sb[:], qt32_d[:])
                    nc.scalar.dma_start(ones_sb[:], ones_d[:])
                    nc.scalar.dma_start(sed_sb[:], sed_d[:])
                    # PSUM [p, (i, t, s)] -> SBUF [p, (s, t, i)] fp16, one
                    # AP-permuted copy per bank spread across engines
                    o16v = opd16[:].rearrange("p (s t i) -> p s t i", s=SPC, t=2)
                    for q in range(4):
                        src = ops[:, 512 * q:512 * (q + 1)].rearrange(
                            "p (i t s) -> p s t i", t=2, s=SPC)
                        dst = o16v[:, :, :, 64 * q:64 * (q + 1)]
                        if q % 2 == 0:
                            nc.vector.tensor_copy(dst, src)
                        else:
                            nc.scalar.copy(dst, src)

                # ---- main loop, software-pipelined over bins:
                # iteration jj emits elementwise(jj), stage1(jj-1),
                # stage2+tail(jj-2) so no engine queue head-of-line blocks
                with tc.tile_pool(name="elw", bufs=2) as elw, \
                     tc.tile_pool(name="usb", bufs=2) as usbp, \
                     tc.tile_pool(name="sqp", bufs=2) as sqp, \
                     tc.tile_pool(name="tailp", bufs=2) as tailp, \
                     tc.tile_pool(name="u_ps", bufs=1, space="PSUM") as u_ps, \
                     tc.tile_pool(name="a_ps", bufs=2, space="PSUM") as a_ps:
                    state = {}

                    def emit_elw(j):
                        lam = LAM32[j]
                        kj = KVAL[j]
                        # fp16-magic range reduction, batched over the 4
                        # stars: th = opd - lam*round(opd/lam) in
                        # [-lam/2, lam/2]; av = |th| lets the cos plane use
                        # the same [-pi, pi] Sin table
                        # ts/tt ops run DVE fast modes (4x/2x); stt has
                        # none.  nlr/nth share the r16/rr tag rings.
                        eng0 = nc.vector if j < 3 else nc.gpsimd
                        r16 = elw.tile([128, SPC * 512], F16, tag="r16")
                        eng0.tensor_scalar(r16[:], opd16[:], 1.0 / lam,
                                           MAGIC, op0=ALU.mult, op1=ALU.add)
                        pp = elw.tile([128, SPC * 1024], F16,
                                      name=f"pp_{j}", tag="pp")
                        nlr = elw.tile([128, SPC * 512], F16, tag="nlr")
                        nc.vector.tensor_scalar(nlr[:], r16[:], -lam,
                                                lam * MAGIC, op0=ALU.mult,
                                                op1=ALU.add)
                        th = elw.tile([128, SPC * 512], F16, tag="th")
                        nc.vector.tensor_tensor(th[:], opd16[:], nlr[:],
                                                op=ALU.add)
                        nth = elw.tile([128, SPC * 512], F16, tag="rr")
                        nc.vector.tensor_scalar(nth[:], th[:],
                                                -1.0, None, op0=ALU.mult)
                        av = elw.tile([128, SPC * 512], F16, tag="av")
                        nc.vector.tensor_tensor(av[:], th[:], nth[:],
                                                op=ALU.max)
                        nc.scalar.activation(pp[:, 0:2048], av[:], AF.Sin,
                                             bias=halfpi[:], scale=-kj)
                        nc.scalar.activation(pp[:, 2048:4096], th[:], AF.Sin,
                                             bias=0.0, scale=kj)
                        state[j] = {"pp": pp}

                    def emit_stage1(j):
                        # stage 1: U = P E^T per star into u_ps; star s at
                        # column 512*(s//2) + 192*(s%2) (bank-aligned)
                        pp = state[j]["pp"]
                        ups = [u_ps.tile([128, 1024], F32,
                                         name=f"ups{_t}_{j}", tag=f"ups{_t}")
                               for _t in range(2)]
                        usb = [usbp.tile([128, SPC * 192], F16,
                                         name=f"usb{t}_{j}", tag=f"u{t}")
                               for t in range(2)]
                        s1 = slice(j * 192, (j + 1) * 192)
                        for s in range(SPC):
                            u0 = 512 * (s // 2) + 192 * (s % 2)
                            for xt in range(2):
                                out = ups[xt][:, u0:u0 + 192]
                                for yi, yt in enumerate((0, 1)):
                                    prs = pp[:, 512 * s + 256 * yt + 128 * xt:
                                             512 * s + 256 * yt + 128 * (xt + 1)]
                                    pis = pp[:, 2048 + 512 * s + 256 * yt + 128 * xt:
                                             2048 + 512 * s + 256 * yt + 128 * (xt + 1)]
                                    nc.tensor.matmul(out, prs,
                                                     taba_sb[yt][:, s1],
                                                     start=(yi == 0),
                                                     stop=False)
                                    nc.tensor.matmul(out, pis,
                                                     tabb_sb[yt][:, s1],
                                                     start=False,
                                                     stop=(yi == 1))
                        # PSUM->SBUF fp16 copies: usb1 on DVE, usb0 split
                        # 2/3 Act + 1/3 DVE to balance engine load
                        src0 = ups[0][:].rearrange(
                            "p (s2 q) -> p s2 q", s2=2)
                        dst0 = usb[0][:].rearrange(
                            "p (s2 r) -> p s2 r", s2=2)
                        nc.scalar.copy(dst0[:, :, 0:288], src0[:, :, 0:288])
                        nc.vector.tensor_copy(dst0[:, :, 288:384],
                                              src0[:, :, 288:384])
                        src1 = ups[1][:].rearrange(
                            "p (s2 q) -> p s2 q", s2=2)[:, :, 0:384]
                        dst1 = usb[1][:].rearrange(
                            "p (s2 r) -> p s2 r", s2=2)
                        nc.vector.tensor_copy(dst1, src1)
                        state[j]["usb"] = usb

                    def emit_stage2(j):
                        # stage 2: A = E U + D; lhsT tables are column
                        # slices of taba/tabb (C, S, -S); D injected via
                        # identity-rhs matmuls from the hi/lo fp16 tables
                        usb = state[j]["usb"]
                        a_all = a_ps.tile([128, SPC * 256], F32,
                                          name=f"a_{j}", tag="a")
                        s2p = slice(j * 128, (j + 1) * 128)
                        ctab = [cpad_sb[t][:, s2p] for t in range(2)]
                        stab = [spad_sb[t][:, s2p] for t in range(2)]
                        nstab = [nspad_sb[t][:, s2p] for t in range(2)]
                        dre = slice((2 * j) * CROP, (2 * j + 1) * CROP)
                        dim = slice((2 * j + 1) * CROP, (2 * j + 2) * CROP)
                        for s in range(SPC):
                            a_s = a_all[:, 256 * s:256 * s + 192]
                            are = a_all[:, 256 * s:256 * s + 96]
                            aim = a_all[:, 256 * s + 96:256 * s + 192]
                            uboth = [usb[xt][:, 192 * s:192 * (s + 1)]
                                     for xt in range(2)]
                            ur = [usb[xt][:, 192 * s:192 * s + 96]
                                  for xt in range(2)]
                            ui = [usb[xt][:, 192 * s + 96:192 * (s + 1)]
                                  for xt in range(2)]
                            islc = i4_sb[:, s * CROP:(s + 1) * CROP]
                            nc.tensor.matmul(a_s, ctab[0], uboth[0],
                                             start=True, stop=False)
                            nc.tensor.matmul(a_s, ctab[1], uboth[1],
                                             start=False, stop=False)
                            nc.tensor.matmul(are, stab[0], ui[0],
                                             start=False, stop=False)
                            nc.tensor.matmul(are, stab[1], ui[1],
                                             start=False, stop=False)
                            nc.tensor.matmul(aim, nstab[0], ur[0],
                                             start=False, stop=False)
                            nc.tensor.matmul(aim, nstab[1], ur[1],
                                             start=False, stop=False)
                            nc.tensor.matmul(are[0:CROP, :],
                                             dthi_sb[:, dre], islc,
                                             start=False, stop=False)
                            nc.tensor.matmul(aim[0:CROP, :],
                                             dthi_sb[:, dim], islc,
                                             start=False, stop=True)
                        state[j]["a"] = a_all

                    def emit_tail(j):
                        # ---- bin tail (batched over the 4 stars) ----
                        a_all = state[j]["a"]
                        sq = sqp.tile([CROP, SPC * 192], F32, tag="sq")
                        av4 = a_all[0:CROP, :].rearrange("p (s g) -> p s g",
                                                         g=256)
                        nc.scalar.activation(
                            sq[:].rearrange("p (s g) -> p s g", g=192),
                            av4[:, :, 0:192], AF.Square)
                        ps_all = sqp.tile([CROP, SPC * 96], F32, tag="ps")
                        sq4 = sq[:].rearrange("p (s h g) -> p s h g", h=2, g=96)
                        nc.gpsimd.tensor_tensor(
                            ps_all[:].rearrange("p (s g) -> p s g", g=96),
                            sq4[:, :, 0, :], sq4[:, :, 1, :], op=ALU.add)
                        ps1 = tailp.tile([CROP, 132], F32, tag="ps1")
                        pv = ps_all[:].rearrange("p (s q c) -> p s q c",
                                                 q=32, c=3)
                        t1 = tailp.tile([CROP, 128], F32, tag="t1")
                        nc.gpsimd.tensor_tensor(
                            t1[:].rearrange("p (s q) -> p s q", q=32),
                            pv[:, :, :, 0], pv[:, :, :, 1], op=ALU.add)
                        nc.gpsimd.tensor_tensor(
                            ps1[:, 0:128].rearrange("p (s q) -> p s q", q=32),
                            t1[:].rearrange("p (s q) -> p s q", q=32),
                            pv[:, :, :, 2], op=ALU.add)
                        nc.vector.tensor_reduce(
                            ps1[:, 128:132],
                            ps1[:, 0:128].rearrange("p (s q) -> p s q", s=SPC),
                            axis=mybir.AxisListType.X, op=ALU.add)
                        # pooling + totals matmuls land in the spare PSUM
                        # columns of this bin's a tile (cols 192.. of the
                        # star blocks)
                        plp = [a_all[0:32, 192:256], a_all[0:32, 448:512]]
                        totp = a_all[0:32, 704:704 + SPC]
                        nc.tensor.matmul(plp[0], qt32_sb[:], ps1[:, 0:64],
                                         start=True, stop=True)
                        nc.tensor.matmul(plp[1], qt32_sb[:], ps1[:, 64:128],
                                         start=True, stop=True)
                        nc.tensor.matmul(totp, ones_sb[:], ps1[:, 128:132],
                                         start=True, stop=True)
                        plsb = tailp.tile([32, 128], F32, tag="plsb")
                        psrc = a_all[0:32, 192:704].rearrange(
                            "p (h c) -> p h c", c=256)[:, :, 0:64]
                        nc.vector.tensor_copy(
                            plsb[:].rearrange("p (h c) -> p h c", c=64), psrc)
                        rcp = tailp.tile([32, SPC], F32, tag="rcp")
                        nc.vector.reciprocal(rcp[:], totp)
                        scl = tailp.tile([32, SPC], F32, tag="scl")
                        nc.gpsimd.tensor_tensor(
                            scl[:], rcp[:], sed_sb[:, j * SPC:(j + 1) * SPC],
                            op=ALU.mult)
                        sclb = scl[:].rearrange(
                            "p (s o) -> p s o", o=1).broadcast_to((32, SPC, 32))
                        tmp = tailp.tile([32, SPC * 32], F32, tag="tmp")
                        nc.gpsimd.tensor_tensor(
                            tmp[:].rearrange("p (s q) -> p s q", q=32),
                            plsb[:].rearrange("p (s q) -> p s q", q=32),
                            sclb, op=ALU.mult)
                        nc.gpsimd.tensor_tensor(psf_all[:], psf_all[:],
                                                tmp[:], op=ALU.add)
                        del state[j]

                    for jj in range(N_BINS + 2):
                        if 1 <= jj < N_BINS + 1:
                            emit_stage1(jj - 1)
                        if 2 <= jj:
                            emit_stage2(jj - 2)
                        if jj < N_BINS:
                            emit_elw(jj)
                        if 2 <= jj:
                            emit_tail(jj - 2)

                    for s in range(SPC):
                        nc.gpsimd.dma_start(psf_out[s],
                                            psf_all[:, 32 * s:32 * (s + 1)])

    nc.compile()
    return nc


_NC_CACHE = []


def _make_in_maps(prep):
    (C, Wt, taba, tabb, cpad, spad, nspad, dthi, i4, qt32, ones96,
     sed) = prep
    shared = {
        "wmat": Wt, "taba": taba, "tabb": tabb, "cpad": cpad, "spad": spad,
        "nspad": nspad, "dthi": dthi, "i4": i4, "qt32": qt32,
        "ones96": ones96,
    }
    in_maps = []
    for c in range(N_CORES):
        sl = slice(c * SPC, (c + 1) * SPC)
        sed_row = np.broadcast_to(
            sed[sl].T.reshape(1, N_BINS * SPC), (32, N_BINS * SPC))
        sed_row = np.ascontiguousarray(sed_row).astype(np.float32)
        in_maps.append(dict(
            shared,
            cmat=np.ascontiguousarray(C[sl].T).astype(np.float16),
            sed=sed_row,
        ))
    return in_maps


def kernel(**inputs):
    in_maps = _make_in_maps(_host_prep(**inputs))

    if not _NC_CACHE:
        _NC_CACHE.append(_build_nc())
    nc = _NC_CACHE[0]

    res = run_bass_kernel_spmd(nc, in_maps, core_ids=list(range(N_CORES)))
    out = np.concatenate([r["psf_out"] for r in res.results], axis=0)
    return out.astype(np.float32)
